# revision 53
# baseline (speedup 1.0000x reference)
"""Trainium2 Bass kernel for 12-head attention (SEQ=4096, D=768), 8-core SPMD.

Sharding: head-parallel with a sequence-split remainder. Core c owns full head
A_c = c and the half of head B_c = 8 + c//2 selected by (c % 2). Upper-half
cores receive a sequence-rolled copy of x so every core's program computes
local queries [0, 2048) for its B head (pure SPMD, no divergent control flow).
Each core returns a partial output projection [768, 4096] (fp16); the host
un-rolls, transposes and sums the 8 partials.

Per-core dataflow (matmuls in fp16, softmax internals in fp32):
  x^T -> QKV^T projections -> scores S^T[j,i] = K^T(lhsT) x Q^T(rhs), the two
  heads of a pair computed concurrently on disjoint PE row groups
  -> exp on ScalarE (scale=1/8 folded; no max subtraction, scores in [-10,10])
  -> attn@V with [V | ones] stationary so softmax denominators fall out as an
  extra PSUM row -> normalize via K=2 broadcast matmul + fast reciprocal ->
  output projection.

Schedule: the ScalarE exp stream (192 x [128,1024] ACTIVATEs ~ 214 us) is the
critical resource; everything else hides under it:
  - first x chunks arrive as [128,512] transfers on both queues; projections
    interleave with the DMA emission so the first exp fires ~21 us in. Pair 0
    needs no shifted q2/k2 copies (B-head rows live at partitions 64:128 of
    q_t/k_t already); the k2 shift-DMAs for pair 1 run late in pair 0.
    NOTE: Tile dependencies are emission-order-forward only — every consumer
    must be emitted after its producers (verify with CoreSim, simcheck.py).
  - per key block the emission order is scores(jb)+exp(jb), injected pieces,
    THEN attn@V(jb-1): the TensorE FIFO stall on exp(jb-1) can never starve
    scores(jb). e tiles (12 bufs) absorb the attn@V lag.
  - pair boundaries pre-emit 3 blocks of next-pair scores/exp around the
    accumulator drains so the exp stream runs through the boundary.
  - phase 3 (denominator broadcast, normalize, output projection, store) for
    chunks 0,1 / 4,5 rides the freed attn@V banks during the next pair;
    chunks 6,7 ride the scores-PSUM rotation late in pair 2; only chunks 2,3
    drain at the end, with stores split across both DMA queues.
"""

import numpy as np

N_CORES = 8
N_HEADS = 12
HEAD_DIM = 64
N_FEATS = 768
SEQ = 4096
FCH = N_FEATS // 128  # contraction chunks of the feature dim
W = 1024              # i-chunk width (exp granularity)
NIC = SEQ // W
NJB = SEQ // 128      # key blocks
NH = W // 512         # 512-wide matmul sub-chunks per i-chunk
NCH = SEQ // 512

_PROGRAM = None
LAST_RESULT = None

import os as _os
AV_LAG = _os.environ.get("KBUG_AV_LAG", "1") == "1"


def _build_program():
    import concourse.tile as tile
    from concourse import bacc, mybir

    f32 = mybir.dt.float32
    f32r = mybir.dt.float32r
    f16 = mybir.dt.float16
    EXP = mybir.ActivationFunctionType.Exp
    MULT = mybir.AluOpType.mult
    ADD = mybir.AluOpType.add

    nc = bacc.Bacc("TRN2", target_bir_lowering=False, debug=False,
                   num_devices=N_CORES)

    xt_d = nc.dram_tensor("xt", [N_FEATS, SEQ], f16, kind="ExternalInput").ap()
    wqk_d = nc.dram_tensor("wqk", [N_FEATS, 256], f16, kind="ExternalInput").ap()
    wv_d = nc.dram_tensor("wv", [N_FEATS, 128], f16, kind="ExternalInput").ap()
    wo_d = nc.dram_tensor("wo", [128, N_FEATS], f16, kind="ExternalInput").ap()
    sel_d = nc.dram_tensor("sel", [2, 128], f32r, kind="ExternalInput").ap()
    id_d = nc.dram_tensor("ident", [128, 128], f16, kind="ExternalInput").ap()
    out_d = nc.dram_tensor("out", [N_FEATS, SEQ], f16, kind="ExternalOutput").ap()

    with tile.TileContext(nc) as tc:
        with tc.tile_pool(name="persist", bufs=1) as pp, \
             tc.tile_pool(name="ps_s", space="PSUM", bufs=2) as ps_s, \
             tc.tile_pool(name="ps_ov", space="PSUM", bufs=1) as ps_ov, \
             tc.tile_pool(name="exps", bufs=12) as pe, \
             tc.tile_pool(name="ph3", bufs=2) as p3:

            # ---- persistent tiles ----
            wqk_sb = pp.tile([128, FCH, 256], f16)
            wv_sb = pp.tile([128, FCH, 128], f16)
            wo_sb = pp.tile([128, N_FEATS], f16)
            sel_sb = pp.tile([66, 128], f32r)
            id_sb = pp.tile([128, 128], f16)
            dummy = pp.tile([128, 16], f32)
            scratch = pp.tile([128, 128], f32)
            warm_src = pp.tile([128, 128], f16)
            q_t = [pp.tile([128, 512], f16, name=f"q_t{i}") for i in range(NCH)]
            k_t = [pp.tile([128, 512], f16, name=f"k_t{i}") for i in range(NCH)]
            q2_t = [pp.tile([128, 512], f16, name=f"q2_t{i}") for i in range(NCH)]
            k2_t = [pp.tile([128, 512], f16, name=f"k2_t{i}") for i in range(NCH)]
            vA_t = pp.tile([128, NJB, 65], f16)
            vB_t = pp.tile([128, NJB, 66], f16)

            xt = pp.tile([128, FCH, SEQ], f16)
            attn_out = pp.tile([128, SEQ], f32)  # rows 0-63 A dims, 64-127 B dims
            den = pp.tile([66, SEQ], f32r)       # rows 64 (A), 65 (B)

            # exp table preload: first ACTIVATE triggers ACT_TABLE_LOAD early
            nc.vector.memset(dummy[:], 0.0)
            nc.scalar.activation(out=dummy[:, 8:16], in_=dummy[:, 0:8], func=EXP)
            nc.vector.memset(warm_src[:], 0.0)

            # PE warm-up on a DMA-independent source: spans the x DMA wait so
            # the first projections run at the warm clock
            wm = ps_s.tile([128, 128], f32, tag="s", name="warm")
            for i in range(32):
                nc.tensor.matmul(wm[:], warm_src[:], warm_src[:],
                                 start=(i == 0), stop=(i == 31))
            nc.vector.tensor_copy(scratch[:], wm[:])

            # ---- input DMAs, first-needed-first ----
            xt_r = xt_d.rearrange("(c p) n -> p c n", p=128)

            def xdma(eng, k, lo, hi):
                eng.dma_start(out=xt[:, k, lo:hi], in_=xt_r[:, k, lo:hi])

            # chunk 0 (cols 0:512) and chunk 1 split across both queues;
            # wqk ahead of x on sync (critical for the first projection), wv
            # behind chunk 0 on gpsimd (first needed by the jb0 v-piece)
            nc.sync.dma_start(out=wqk_sb[:],
                              in_=wqk_d.rearrange("(c p) m -> p c m", p=128))
            nc.gpsimd.dma_start(out=id_sb[:], in_=id_d[:])
            for k in range(3):
                xdma(nc.sync, k, 0, 512)
            for k in range(3, FCH):
                xdma(nc.gpsimd, k, 0, 512)
            nc.gpsimd.dma_start(out=wv_sb[:],
                                in_=wv_d.rearrange("(c p) m -> p c m", p=128))
            for k in range(3):
                xdma(nc.sync, k, 512, 1024)
            for k in range(3, FCH):
                xdma(nc.gpsimd, k, 512, 1024)

            # ---- projection streams ----
            def proj_qk(nch):
                pq = ps_s.tile([128, W], f32, tag="s", name=f"pjqk{nch}")
                for k in range(FCH):
                    nc.tensor.matmul(pq[:, 0:512], wqk_sb[:, k, 0:128],
                                     xt[:, k, nch * 512:(nch + 1) * 512],
                                     start=(k == 0), stop=(k == FCH - 1))
                    nc.tensor.matmul(pq[:, 512:1024], wqk_sb[:, k, 128:256],
                                     xt[:, k, nch * 512:(nch + 1) * 512],
                                     start=(k == 0), stop=(k == FCH - 1))
                nc.vector.tensor_copy(q_t[nch][:], pq[:, 0:512])
                nc.vector.tensor_copy(k_t[nch][:], pq[:, 512:1024])

            def piece_vd(b):
                # transposed V projection: xt slice as lhsT puts j on the out
                # partitions, so no PE transpose is needed at all
                pv = ps_s.tile([128, 128], f32, tag="s", name=f"pvd{b}")
                for k in range(FCH):
                    nc.tensor.matmul(pv[:], xt[:, k, b * 128:(b + 1) * 128],
                                     wv_sb[:, k, :],
                                     start=(k == 0), stop=(k == FCH - 1))
                nc.vector.tensor_copy(vA_t[:, b, 0:64], pv[:, 0:64])
                nc.vector.tensor_copy(vB_t[:, b, 0:64], pv[:, 64:128])

            # 3-matmul pieces: PSUM partial + vector-engine combine
            part_state = {}

            def _half_p1(c, key, wt, lo, hi):
                ps = ps_s.tile([128, 512], f32, tag="s", name=f"pp1{key}{c}")
                for k in range(3):
                    nc.tensor.matmul(ps[:], wt[:, k, lo:hi],
                                     xt[:, k, c * 512:(c + 1) * 512],
                                     start=(k == 0), stop=(k == 2))
                pt = p3.tile([128, 512], f32, tag=f"part_{key}", name=f"pt{key}{c}", bufs=2)
                nc.vector.tensor_copy(pt[:], ps[:])
                part_state[(key, c)] = pt

            def _half_p2(c, key, wt, lo, hi, dest):
                ps = ps_s.tile([128, 512], f32, tag="s", name=f"pp2{key}{c}")
                for k in range(3, FCH):
                    nc.tensor.matmul(ps[:], wt[:, k, lo:hi],
                                     xt[:, k, c * 512:(c + 1) * 512],
                                     start=(k == 3), stop=(k == FCH - 1))
                nc.vector.tensor_tensor(out=dest[:], in0=ps[:],
                                        in1=part_state.pop((key, c))[:], op=ADD)

            def piece_k1(c): _half_p1(c, "k", wqk_sb, 128, 256)

            def piece_k2(c):
                _half_p2(c, "k", wqk_sb, 128, 256, k_t[c])
                nc.gpsimd.dma_start(out=k2_t[c][64:128, :], in_=k_t[c][0:64, :])

            def piece_q1(c): _half_p1(c, "q", wqk_sb, 0, 128)

            def piece_q2(c):
                _half_p2(c, "q", wqk_sb, 0, 128, q_t[c])
                nc.gpsimd.dma_start(out=q2_t[c][64:128, :], in_=q_t[c][0:64, :])

            def piece_k2dma(c):
                # pair-1 lhsT shift copies for the prologue chunks
                nc.gpsimd.dma_start(out=k2_t[c][64:128, :], in_=k_t[c][0:64, :])

            dmaq = [0]

            def _store(ob, fb, t0):
                eng = nc.sync if dmaq[0] % 2 == 0 else nc.gpsimd
                dmaq[0] += 1
                eng.dma_start(out=out_d[fb * 128:(fb + 1) * 128, t0:t0 + 512],
                              in_=ob[:])

            # ---- phase 3 on the scores-PSUM rotation (injected into pairs) ----
            sph3_nm = {}

            def sph3_bcast(t):
                t0 = t * 512
                bc = ps_s.tile([128, 512], f32, tag="s", name=f"sbc{t}")
                nc.tensor.matmul(bc[:], sel_sb[64:66, :],
                                 den[64:66, t0:t0 + 512], start=True, stop=True)
                rc = p3.tile([128, 512], f32, tag="rc", name=f"src{t}", bufs=3)
                nc.vector.reciprocal_approx_fast(out=rc[:], in_=bc[:])
                nm = p3.tile([128, 512], f16, tag="nm", name=f"snm{t}", bufs=5)
                nc.vector.tensor_tensor(out=nm[:], in0=attn_out[:, t0:t0 + 512],
                                        in1=rc[:], op=MULT)
                sph3_nm[t] = nm

            def sph3_step(t, fb):
                t0 = t * 512
                po = ps_s.tile([128, 512], f32, tag="s", name=f"spo{t}_{fb}")
                nc.tensor.matmul(po[:], wo_sb[:, fb * 128:(fb + 1) * 128],
                                 sph3_nm[t][:], start=True, stop=True)
                ob = p3.tile([128, 512], f16, tag="ob", name=f"sob{t}_{fb}", bufs=6)
                nc.vector.tensor_copy(ob[:], po[:])
                _store(ob, fb, t0)

            pairs = [
                (("A", 0, q_t, k_t, vA_t, 65), ("B", 0, q_t, k_t, vB_t, 66)),
                (("A", 2, q_t, k_t, vA_t, 65), ("A2", 3, q2_t, k2_t, vA_t, 65)),
                (("A", 1, q_t, k_t, vA_t, 65), ("B", 1, q_t, k_t, vB_t, 66)),
            ]

            pre_e = {}

            def _score_e(pi, jb, h, c1, c2):
                jc, jo = jb // 4, (jb % 4) * 128
                sp = ps_s.tile([128, W], f32, tag="s", name=f"s{pi}_{jb}_{h}")
                for ci, (_, ic, qt, kt, _, _) in enumerate((c1, c2)):
                    base = ci * 64
                    nc.tensor.matmul(
                        sp[:, ci * 512:(ci + 1) * 512],
                        kt[jc][base:base + 64, jo:jo + 128],
                        qt[ic * NH + h][base:base + 64, :],
                        start=True, stop=True)
                e = pe.tile([128, W], f16, tag="e", name=f"e{pi}_{jb}_{h}")
                nc.scalar.activation(out=e[:], in_=sp[:], func=EXP, scale=0.125)
                return e

            # ---- prologue: project chunks 0,1 and start the exp stream ----
            proj_qk(0)
            pre_e[(0, 0, 0)] = _score_e(0, 0, 0, *pairs[0])
            proj_qk(1)
            pre_e[(0, 0, 1)] = _score_e(0, 0, 1, *pairs[0])
            pre_e[(0, 1, 0)] = _score_e(0, 1, 0, *pairs[0])
            pre_e[(0, 1, 1)] = _score_e(0, 1, 1, *pairs[0])

            # remaining input DMAs (queue order continues behind the prologue)
            for cp in range(1, SEQ // 1024):
                for k in range(3):
                    xdma(nc.sync, k, cp * 1024, (cp + 1) * 1024)
                for k in range(3, FCH):
                    xdma(nc.gpsimd, k, cp * 1024, (cp + 1) * 1024)
                if cp == 2:
                    nc.gpsimd.dma_start(out=wo_sb[:], in_=wo_d[:])
                    nc.gpsimd.dma_start(out=sel_sb[64:66, :], in_=sel_d[:])

            # bulk constants
            nc.vector.memset(vA_t[:, :, 64:65], 1.0)
            nc.vector.memset(vB_t[:, :, 64:65], 0.0)
            nc.vector.memset(vB_t[:, :, 65:66], 1.0)
            nc.vector.memset(den[64:66, SEQ // 2:].bitcast(f32), 1.0)
            nc.vector.memset(attn_out[64:128, SEQ // 2:], 0.0)



            # injected work per (pair, key block)
            inj = [
                {
                    0: [(piece_vd, 0)],
                    1: [(piece_vd, 1)],
                    2: [(piece_vd, 2)],
                    3: [(piece_vd, 3)],
                    4: [(piece_vd, 4), (piece_k1, 2)],
                    5: [(piece_vd, 5), (piece_k2, 2)],
                    6: [(piece_vd, 6), (piece_k1, 3)],
                    7: [(piece_vd, 7), (piece_k2, 3)],
                    8: [(piece_vd, 8), (piece_k1, 4)],
                    9: [(piece_vd, 9), (piece_k2, 4)],
                    10: [(piece_vd, 10), (piece_k1, 5)],
                    11: [(piece_vd, 11), (piece_k2, 5)],
                    12: [(piece_vd, 12), (piece_k1, 6)],
                    13: [(piece_vd, 13), (piece_k2, 6)],
                    14: [(piece_vd, 14), (piece_k1, 7)],
                    15: [(piece_vd, 15), (piece_k2, 7)],
                    16: [(piece_vd, 16), (piece_q1, 6)],
                    17: [(piece_vd, 17), (piece_q2, 6)],
                    18: [(piece_vd, 18), (piece_q1, 7)],
                    19: [(piece_vd, 19), (piece_q2, 7)],
                    20: [(piece_vd, 20), (piece_q1, 4)],
                    21: [(piece_vd, 21), (piece_q2, 4)],
                    22: [(piece_vd, 22), (piece_q1, 5)],
                    23: [(piece_vd, 23), (piece_q2, 5)],
                    24: [(piece_vd, 24), (piece_k2dma, 0)],
                    25: [(piece_vd, 25), (piece_k2dma, 1)],
                    26: [(piece_vd, 26)],
                    27: [(piece_vd, 27)],
                    28: [(piece_vd, 28)],
                    29: [(piece_vd, 29)],
                    30: [(piece_vd, 30)],
                    31: [(piece_vd, 31)],
                },
                {
                    13: [(piece_q1, 2), (piece_q2, 2)],
                    15: [(piece_q1, 3), (piece_q2, 3)],
                },
                {},
            ]

            def _sph3_sched(tbl, chunks, jb0):
                """Schedule bcast + 6 outproj steps per chunk, 2 ops/jb."""
                jb = jb0
                for t in chunks:
                    tbl.setdefault(jb, []).append((sph3_bcast, t))
                    jb += 1
                    for fb0 in (0, 2, 4):
                        ops = tbl.setdefault(jb, [])
                        ops.append((lambda c, t=t, fb=fb0: sph3_step(t, fb), 0))
                        ops.append((lambda c, t=t, fb=fb0 + 1: sph3_step(t, fb), 0))
                        jb += 1

            _sph3_sched(inj[1], [0, 1], 5)
            _sph3_sched(inj[2], [4, 5], 5)
            _sph3_sched(inj[2], [6, 7], 20)

            def emit_av(pi, prev, c1, c2, ov):
                jb, etiles = prev
                for h, e in enumerate(etiles):
                    for ci, (_, ic, _, _, vt, m) in enumerate((c1, c2)):
                        nc.tensor.matmul(
                            ov[0:m, ci * W + h * 512:ci * W + (h + 1) * 512],
                            vt[:, jb, 0:m], e[:, ci * 512:(ci + 1) * 512],
                            start=(jb == 0), stop=(jb == NJB - 1))

            def drain(pi, c1, c2, ov):
                (n1, ic1, _, _, _, _), (n2, ic2, _, _, _, _) = c1, c2
                p10, p20 = ic1 * W, ic2 * W
                ovA, ovB = ov[:, 0:W], ov[:, W:2 * W]
                if n2 == "B":
                    ovb_sb = pe.tile([64, W], f32, tag="ovb_sb", name=f"ovb_sb{pi}", bufs=2)
                    nc.vector.tensor_copy(ovb_sb[:], ovB[0:64, :])
                    nc.gpsimd.dma_start(out=attn_out[64:128, p20:p20 + W], in_=ovb_sb[:])
                    nc.vector.tensor_copy(den[64:66, p20:p20 + W], ovB[64:66, :])
                    nc.vector.tensor_copy(den[64:65, p10:p10 + W], ovA[64:65, :])
                    if pi == len(pairs) - 1:
                        # last pair: ScalarE is idle after the final exp —
                        # give it the attn copy so den feeds the tail sooner
                        nc.scalar.copy(attn_out[0:64, p10:p10 + W], ovA[0:64, :])
                    else:
                        nc.vector.tensor_copy(attn_out[0:64, p10:p10 + W], ovA[0:64, :])
                else:
                    nc.vector.tensor_copy(den[64:65, p10:p10 + W], ovA[64:65, :])
                    nc.vector.tensor_copy(den[64:65, p20:p20 + W], ovB[64:65, :])
                    nc.vector.tensor_copy(attn_out[0:64, p10:p10 + W], ovA[0:64, :])
                    nc.vector.tensor_copy(attn_out[0:64, p20:p20 + W], ovB[0:64, :])

            for pi, (c1, c2) in enumerate(pairs):
                ov = ps_ov.tile([128, 2 * W], f32, tag="ov", name=f"ov_{pi}")
                prev_av = None
                for jb in range(NJB):
                    # scores + exp first: the exp stream must never wait on
                    # attn@V sitting ahead of it in the TensorE FIFO
                    etiles = []
                    for h in range(NH):
                        e = pre_e.pop((pi, jb, h), None)
                        if e is None:
                            e = _score_e(pi, jb, h, c1, c2)
                        etiles.append(e)
                    for fn, c in inj[pi].get(jb, ()):
                        fn(c)
                    if AV_LAG:
                        if prev_av is not None:
                            emit_av(pi, prev_av, c1, c2, ov)
                        prev_av = (jb, etiles)
                    else:
                        emit_av(pi, (jb, etiles), c1, c2, ov)
                        prev_av = None

                # ---- pair boundary: keep the exp stream running through the
                # drain by pre-emitting 3 blocks of next-pair scores/exp ----
                nxt = pairs[pi + 1] if pi + 1 < len(pairs) else None
                if nxt is not None:
                    for h in range(NH):
                        pre_e[(pi + 1, 0, h)] = _score_e(pi + 1, 0, h, *nxt)
                if prev_av is not None:
                    emit_av(pi, prev_av, c1, c2, ov)

                # drain accumulators to SBUF
                drain(pi, c1, c2, ov)

                if nxt is not None:
                    for h in range(NH):
                        pre_e[(pi + 1, 1, h)] = _score_e(pi + 1, 1, h, *nxt)
                    for h in range(NH):
                        pre_e[(pi + 1, 2, h)] = _score_e(pi + 1, 2, h, *nxt)

            # ---- tail: chunks 2,3 (out cols 1024:2048), 1024-wide ops ----
            nm23 = p3.tile([128, 1024], f16, tag="nm23", bufs=1)
            for idx, t in enumerate((2, 3)):
                t0 = t * 512
                bcg = ps_s.tile([128, 512], f32, tag="s", name=f"tbc{t}")
                nc.tensor.matmul(bcg[:], sel_sb[64:66, :], den[64:66, t0:t0 + 512],
                                 start=True, stop=True)
                rc = p3.tile([128, 512], f32, tag="rc", name=f"trc{t}", bufs=3)
                nc.vector.reciprocal_approx_fast(out=rc[:], in_=bcg[:])
                nc.vector.tensor_tensor(out=nm23[:, idx * 512:(idx + 1) * 512],
                                        in0=attn_out[:, t0:t0 + 512],
                                        in1=rc[:], op=MULT)
            ovt = ps_ov.tile([128, 2 * W], f32, tag="ov", name="tailov")
            for fb in range(6):
                if fb % 2 == 0:
                    po = ps_s.tile([128, 1024], f32, tag="s", name=f"tpo{fb}")
                else:
                    half = (fb // 2) % 2
                    po = ovt[:, half * 1024:(half + 1) * 1024]
                nc.tensor.matmul(po[:, 0:512], wo_sb[:, fb * 128:(fb + 1) * 128],
                                 nm23[:, 0:512], start=True, stop=True)
                nc.tensor.matmul(po[:, 512:1024], wo_sb[:, fb * 128:(fb + 1) * 128],
                                 nm23[:, 512:1024], start=True, stop=True)
                ob = p3.tile([128, 1024], f16, tag="ob2", name=f"tob{fb}", bufs=4)
                if fb % 2 == 1:
                    nc.scalar.copy(ob[:], po[:, 0:1024])
                else:
                    nc.vector.tensor_copy(ob[:], po[:, 0:1024])
                eng = nc.sync if fb % 2 == 0 else nc.gpsimd
                eng.dma_start(out=out_d[fb * 128:(fb + 1) * 128, 1024:2048],
                              in_=ob[:])

    nc.compile()
    return nc


def _get_program():
    global _PROGRAM
    if _PROGRAM is None:
        _PROGRAM = _build_program()
    return _PROGRAM


def kernel(x: np.ndarray, w_qkv: np.ndarray, w_out: np.ndarray) -> np.ndarray:
    global LAST_RESULT
    import os
    try:
        import antenv.axon_hooks  # noqa: F401
    except ImportError:
        # without the NTFF hook, a leaked BASS_TRACE=1 would crash the
        # axon trace path inside run_bass_kernel_spmd
        os.environ["BASS_NEVER_TRACE"] = "1"
    from concourse.bass_utils import run_bass_kernel_spmd

    nc = _get_program()
    x2 = np.ascontiguousarray(x[0], dtype=np.float32)          # [SEQ, F]
    w_qkv = np.asarray(w_qkv, dtype=np.float32)                # [2304, F]
    w_out = np.asarray(w_out, dtype=np.float32)                # [F, 768]

    # per-head slices of w_qkv rows: o = h*192 + d*3 + {0:q, 1:k, 2:v}
    def wslice(h, which):
        return w_qkv[h * 192 + which:(h + 1) * 192:3, :]       # [64, F]

    ident = np.eye(128, dtype=np.float16)
    sel = np.zeros((2, 128), dtype=np.float32)
    sel[0, 0:64] = 1.0
    sel[1, 64:128] = 1.0

    xt_plain = np.ascontiguousarray(x2.T.astype(np.float16))   # [F, SEQ]
    xt_rolled = np.ascontiguousarray(np.roll(x2, -SEQ // 2, axis=0).T.astype(np.float16))

    in_maps = []
    rolls = []
    for c in range(N_CORES):
        hA = c
        hB = 8 + c // 2
        roll = (SEQ // 2) if (c % 2) else 0
        rolls.append(roll)
        wqk = np.ascontiguousarray(np.concatenate(
            [wslice(hA, 0), wslice(hB, 0), wslice(hA, 1), wslice(hB, 1)],
            axis=0).T.astype(np.float16))
        wv = np.ascontiguousarray(np.concatenate(
            [wslice(hA, 2), wslice(hB, 2)], axis=0).T.astype(np.float16))
        cols = list(range(hA * 64, hA * 64 + 64)) + list(range(hB * 64, hB * 64 + 64))
        wo = np.ascontiguousarray(w_out[:, cols].T.astype(np.float16))  # [128, F]
        in_maps.append({
            "xt": xt_rolled if roll else xt_plain,
            "wqk": wqk, "wv": wv, "wo": wo, "sel": sel, "ident": ident,
        })

    res = run_bass_kernel_spmd(nc, in_maps, list(range(N_CORES)))
    LAST_RESULT = res

    acc = np.zeros((SEQ, N_FEATS), dtype=np.float64)
    for c in range(N_CORES):
        part = res.results[c]["out"]                           # [F, SEQ] fp16
        if rolls[c]:
            part = np.roll(part, rolls[c], axis=1)
        acc += part.T.astype(np.float64)
    return acc.astype(np.float32)[None]


# revision 54
# speedup vs baseline: 1.0223x; 1.0223x over previous
"""Trainium2 Bass kernel for 12-head attention (SEQ=4096, D=768), 8-core SPMD.

Sharding: head-parallel with a sequence-split remainder. Core c owns full head
A_c = c and the half of head B_c = 8 + c//2 selected by (c % 2). Upper-half
cores receive a sequence-rolled copy of x so every core's program computes
local queries [0, 2048) for its B head (pure SPMD, no divergent control flow).
Each core returns a partial output projection [768, 4096] (fp16); the host
un-rolls, transposes and sums the 8 partials.

Per-core dataflow (matmuls in fp16, softmax internals in fp32):
  x^T -> QKV^T projections -> scores S^T[j,i] = K^T(lhsT) x Q^T(rhs), the two
  heads of a pair computed concurrently on disjoint PE row groups
  -> exp on ScalarE (scale=1/8 folded; no max subtraction, scores in [-10,10])
  -> attn@V with [V | ones] stationary so softmax denominators fall out as an
  extra PSUM row -> normalize via K=2 broadcast matmul + fast reciprocal ->
  output projection.

Schedule: the ScalarE exp stream (192 x [128,1024] ACTIVATEs ~ 214 us) is the
critical resource; everything else hides under it:
  - first x chunks arrive as [128,512] transfers on both queues; projections
    interleave with the DMA emission so the first exp fires ~21 us in. Pair 0
    needs no shifted q2/k2 copies (B-head rows live at partitions 64:128 of
    q_t/k_t already); the k2 shift-DMAs for pair 1 run late in pair 0.
    NOTE: Tile dependencies are emission-order-forward only — every consumer
    must be emitted after its producers (verify with CoreSim, simcheck.py).
  - per key block the emission order is scores(jb)+exp(jb), injected pieces,
    THEN attn@V(jb-1): the TensorE FIFO stall on exp(jb-1) can never starve
    scores(jb). e tiles (12 bufs) absorb the attn@V lag.
  - pair boundaries pre-emit 3 blocks of next-pair scores/exp around the
    accumulator drains so the exp stream runs through the boundary.
  - phase 3 (denominator broadcast, normalize, output projection, store) for
    chunks 0,1 / 4,5 rides the freed attn@V banks during the next pair;
    chunks 6,7 ride the scores-PSUM rotation late in pair 2; only chunks 2,3
    drain at the end, with stores split across both DMA queues.
"""

import numpy as np

N_CORES = 8
N_HEADS = 12
HEAD_DIM = 64
N_FEATS = 768
SEQ = 4096
FCH = N_FEATS // 128  # contraction chunks of the feature dim
W = 1024              # i-chunk width (exp granularity)
NIC = SEQ // W
NJB = SEQ // 128      # key blocks
NH = W // 512         # 512-wide matmul sub-chunks per i-chunk
NCH = SEQ // 512

_PROGRAM = None
LAST_RESULT = None

import os as _os
AV_LAG = _os.environ.get("KBUG_AV_LAG", "1") == "1"


def _build_program():
    import concourse.tile as tile
    from concourse import bacc, mybir

    f32 = mybir.dt.float32
    f32r = mybir.dt.float32r
    f16 = mybir.dt.float16
    EXP = mybir.ActivationFunctionType.Exp
    MULT = mybir.AluOpType.mult
    ADD = mybir.AluOpType.add

    nc = bacc.Bacc("TRN2", target_bir_lowering=False, debug=False,
                   num_devices=N_CORES)

    xt_d = nc.dram_tensor("xt", [N_FEATS, SEQ], f16, kind="ExternalInput").ap()
    wqk_d = nc.dram_tensor("wqk", [N_FEATS, 256], f16, kind="ExternalInput").ap()
    wv_d = nc.dram_tensor("wv", [N_FEATS, 128], f16, kind="ExternalInput").ap()
    wo_d = nc.dram_tensor("wo", [128, N_FEATS], f16, kind="ExternalInput").ap()
    sel_d = nc.dram_tensor("sel", [2, 128], f32r, kind="ExternalInput").ap()
    id_d = nc.dram_tensor("ident", [128, 128], f16, kind="ExternalInput").ap()
    out_d = nc.dram_tensor("out", [N_FEATS, SEQ], f16, kind="ExternalOutput").ap()

    with tile.TileContext(nc) as tc:
        with tc.tile_pool(name="persist", bufs=1) as pp, \
             tc.tile_pool(name="ps_s", space="PSUM", bufs=2) as ps_s, \
             tc.tile_pool(name="ps_ov", space="PSUM", bufs=1) as ps_ov, \
             tc.tile_pool(name="exps", bufs=12) as pe, \
             tc.tile_pool(name="ph3", bufs=2) as p3:

            # ---- persistent tiles ----
            wqk_sb = pp.tile([128, FCH, 256], f16)
            wv_sb = pp.tile([128, FCH, 128], f16)
            wo_sb = pp.tile([128, N_FEATS], f16)
            sel_sb = pp.tile([66, 128], f32r)
            id_sb = pp.tile([128, 128], f16)
            dummy = pp.tile([128, 16], f32)
            scratch = pp.tile([128, 128], f32)
            warm_src = pp.tile([128, 128], f16)
            q_t = [pp.tile([128, 512], f16, name=f"q_t{i}") for i in range(NCH)]
            k_t = [pp.tile([128, 512], f16, name=f"k_t{i}") for i in range(NCH)]
            q2_t = [pp.tile([128, 512], f16, name=f"q2_t{i}") for i in range(NCH)]
            k2_t = [pp.tile([128, 512], f16, name=f"k2_t{i}") for i in range(NCH)]
            vA_t = pp.tile([128, NJB, 65], f16)
            vB_t = pp.tile([128, NJB, 66], f16)
            vT_t = [pp.tile([128, 512], f16, name=f"vT_t{i}") for i in range(NCH)]
            xt = pp.tile([128, FCH, SEQ], f16)
            attn_out = pp.tile([128, SEQ], f32)  # rows 0-63 A dims, 64-127 B dims
            den = pp.tile([66, SEQ], f32r)       # rows 64 (A), 65 (B)

            # exp table preload: first ACTIVATE triggers ACT_TABLE_LOAD early
            nc.vector.memset(dummy[:], 0.0)
            nc.scalar.activation(out=dummy[:, 8:16], in_=dummy[:, 0:8], func=EXP)
            nc.vector.memset(warm_src[:], 0.0)

            # PE warm-up on a DMA-independent source: spans the x DMA wait so
            # the first projections run at the warm clock
            wm = ps_s.tile([128, 128], f32, tag="s", name="warm")
            for i in range(32):
                nc.tensor.matmul(wm[:], warm_src[:], warm_src[:],
                                 start=(i == 0), stop=(i == 31))
            nc.vector.tensor_copy(scratch[:], wm[:])

            # ---- input DMAs, first-needed-first ----
            xt_r = xt_d.rearrange("(c p) n -> p c n", p=128)

            def xdma(eng, k, lo, hi):
                eng.dma_start(out=xt[:, k, lo:hi], in_=xt_r[:, k, lo:hi])

            # chunk 0 (cols 0:512) and chunk 1 split across both queues;
            # wqk ahead of x on sync (critical for the first projection), wv
            # behind chunk 0 on gpsimd (first needed by the jb0 v-piece)
            nc.sync.dma_start(out=wqk_sb[:],
                              in_=wqk_d.rearrange("(c p) m -> p c m", p=128))
            nc.gpsimd.dma_start(out=id_sb[:], in_=id_d[:])
            for k in range(3):
                xdma(nc.sync, k, 0, 512)
            for k in range(3, FCH):
                xdma(nc.gpsimd, k, 0, 512)
            nc.gpsimd.dma_start(out=wv_sb[:],
                                in_=wv_d.rearrange("(c p) m -> p c m", p=128))
            for k in range(3):
                xdma(nc.sync, k, 512, 1024)
            for k in range(3, FCH):
                xdma(nc.gpsimd, k, 512, 1024)

            # ---- projection streams ----
            def proj_qk(nch):
                pq = ps_s.tile([128, W], f32, tag="s", name=f"pjqk{nch}")
                for k in range(FCH):
                    nc.tensor.matmul(pq[:, 0:512], wqk_sb[:, k, 0:128],
                                     xt[:, k, nch * 512:(nch + 1) * 512],
                                     start=(k == 0), stop=(k == FCH - 1))
                    nc.tensor.matmul(pq[:, 512:1024], wqk_sb[:, k, 128:256],
                                     xt[:, k, nch * 512:(nch + 1) * 512],
                                     start=(k == 0), stop=(k == FCH - 1))
                nc.vector.tensor_copy(q_t[nch][:], pq[:, 0:512])
                nc.vector.tensor_copy(k_t[nch][:], pq[:, 512:1024])

            def _trans(nch):
                ptt = ps_s.tile([128, 4, 128], f16, tag="s", name=f"ptr{nch}")
                for q in range(4):
                    nc.tensor.transpose(ptt[:, q, :], vT_t[nch][:, q * 128:(q + 1) * 128], id_sb[:])
                jb0 = nch * 4
                nc.vector.tensor_copy(vA_t[:, jb0:jb0 + 4, 0:64], ptt[:, 0:4, 0:64])
                nc.vector.tensor_copy(vB_t[:, jb0:jb0 + 4, 0:64], ptt[:, 0:4, 64:128])

            # 3-matmul pieces: PSUM partial + vector-engine combine
            part_state = {}

            def _half_p1(c, key, wt, lo, hi):
                ps = ps_s.tile([128, 512], f32, tag="s", name=f"pp1{key}{c}")
                for k in range(3):
                    nc.tensor.matmul(ps[:], wt[:, k, lo:hi],
                                     xt[:, k, c * 512:(c + 1) * 512],
                                     start=(k == 0), stop=(k == 2))
                pt = p3.tile([128, 512], f32, tag=f"part_{key}", name=f"pt{key}{c}", bufs=2)
                nc.vector.tensor_copy(pt[:], ps[:])
                part_state[(key, c)] = pt

            def _half_p2(c, key, wt, lo, hi, dest):
                ps = ps_s.tile([128, 512], f32, tag="s", name=f"pp2{key}{c}")
                for k in range(3, FCH):
                    nc.tensor.matmul(ps[:], wt[:, k, lo:hi],
                                     xt[:, k, c * 512:(c + 1) * 512],
                                     start=(k == 3), stop=(k == FCH - 1))
                nc.vector.tensor_tensor(out=dest[:], in0=ps[:],
                                        in1=part_state.pop((key, c))[:], op=ADD)

            def piece_k1(c): _half_p1(c, "k", wqk_sb, 128, 256)

            def piece_k2(c):
                _half_p2(c, "k", wqk_sb, 128, 256, k_t[c])
                nc.gpsimd.dma_start(out=k2_t[c][64:128, :], in_=k_t[c][0:64, :])

            def piece_q1(c): _half_p1(c, "q", wqk_sb, 0, 128)

            def piece_q2(c):
                _half_p2(c, "q", wqk_sb, 0, 128, q_t[c])
                nc.gpsimd.dma_start(out=q2_t[c][64:128, :], in_=q_t[c][0:64, :])

            def piece_v1(c): _half_p1(c, "v", wv_sb, 0, 128)

            def piece_v2(c): _half_p2(c, "v", wv_sb, 0, 128, vT_t[c])

            def piece_tr(c): _trans(c)

            def piece_k2dma(c):
                # pair-1 lhsT shift copies for the prologue chunks
                nc.gpsimd.dma_start(out=k2_t[c][64:128, :], in_=k_t[c][0:64, :])

            dmaq = [0]

            def _store(ob, fb, t0):
                eng = nc.sync if dmaq[0] % 2 == 0 else nc.gpsimd
                dmaq[0] += 1
                eng.dma_start(out=out_d[fb * 128:(fb + 1) * 128, t0:t0 + 512],
                              in_=ob[:])

            # ---- phase 3 on the scores-PSUM rotation (injected into pairs) ----
            sph3_nm = {}

            def sph3_bcast(t):
                t0 = t * 512
                bc = ps_s.tile([128, 512], f32, tag="s", name=f"sbc{t}")
                nc.tensor.matmul(bc[:], sel_sb[64:66, :],
                                 den[64:66, t0:t0 + 512], start=True, stop=True)
                rc = p3.tile([128, 512], f32, tag="rc", name=f"src{t}", bufs=3)
                nc.vector.reciprocal_approx_fast(out=rc[:], in_=bc[:])
                nm = p3.tile([128, 512], f16, tag="nm", name=f"snm{t}", bufs=5)
                nc.vector.tensor_tensor(out=nm[:], in0=attn_out[:, t0:t0 + 512],
                                        in1=rc[:], op=MULT)
                sph3_nm[t] = nm

            def sph3_step(t, fb):
                t0 = t * 512
                po = ps_s.tile([128, 512], f32, tag="s", name=f"spo{t}_{fb}")
                nc.tensor.matmul(po[:], wo_sb[:, fb * 128:(fb + 1) * 128],
                                 sph3_nm[t][:], start=True, stop=True)
                ob = p3.tile([128, 512], f16, tag="ob", name=f"sob{t}_{fb}", bufs=6)
                nc.vector.tensor_copy(ob[:], po[:])
                _store(ob, fb, t0)

            pairs = [
                (("A", 0, q_t, k_t, vA_t, 65), ("B", 0, q_t, k_t, vB_t, 66)),
                (("A", 2, q_t, k_t, vA_t, 65), ("A2", 3, q2_t, k2_t, vA_t, 65)),
                (("A", 1, q_t, k_t, vA_t, 65), ("B", 1, q_t, k_t, vB_t, 66)),
            ]

            pre_e = {}

            def _score_e(pi, jb, h, c1, c2):
                jc, jo = jb // 4, (jb % 4) * 128
                sp = ps_s.tile([128, W], f32, tag="s", name=f"s{pi}_{jb}_{h}")
                for ci, (_, ic, qt, kt, _, _) in enumerate((c1, c2)):
                    base = ci * 64
                    nc.tensor.matmul(
                        sp[:, ci * 512:(ci + 1) * 512],
                        kt[jc][base:base + 64, jo:jo + 128],
                        qt[ic * NH + h][base:base + 64, :],
                        start=True, stop=True)
                e = pe.tile([128, W], f16, tag="e", name=f"e{pi}_{jb}_{h}")
                nc.scalar.activation(out=e[:], in_=sp[:], func=EXP, scale=0.125)
                return e

            # ---- prologue: project chunks 0,1 and start the exp stream ----
            proj_qk(0)
            pre_e[(0, 0, 0)] = _score_e(0, 0, 0, *pairs[0])
            proj_qk(1)
            pre_e[(0, 0, 1)] = _score_e(0, 0, 1, *pairs[0])
            pre_e[(0, 1, 0)] = _score_e(0, 1, 0, *pairs[0])
            pre_e[(0, 1, 1)] = _score_e(0, 1, 1, *pairs[0])

            # remaining input DMAs (queue order continues behind the prologue)
            for cp in range(1, SEQ // 1024):
                for k in range(3):
                    xdma(nc.sync, k, cp * 1024, (cp + 1) * 1024)
                for k in range(3, FCH):
                    xdma(nc.gpsimd, k, cp * 1024, (cp + 1) * 1024)
                if cp == 2:
                    nc.gpsimd.dma_start(out=wo_sb[:], in_=wo_d[:])
                    nc.gpsimd.dma_start(out=sel_sb[64:66, :], in_=sel_d[:])

            # bulk constants
            nc.vector.memset(vA_t[:, :, 64:65], 1.0)
            nc.vector.memset(vB_t[:, :, 64:65], 0.0)
            nc.vector.memset(vB_t[:, :, 65:66], 1.0)
            nc.vector.memset(den[64:66, SEQ // 2:].bitcast(f32), 1.0)
            nc.vector.memset(attn_out[64:128, SEQ // 2:], 0.0)

            # chunk-0 V projection in the prologue: its transpose feeds the
            # very first attn@V, so it must precede that emission
            piece_v1(0)
            piece_v2(0)

            # injected work per (pair, key block)
            inj = [
                {
                    0: [(piece_tr, 0), (piece_v1, 1)],
                    1: [(piece_v2, 1), (piece_tr, 1)],
                    6: [(piece_k1, 2), (piece_k2, 2)],
                    7: [(piece_v1, 2), (piece_v2, 2)],
                    8: [(piece_tr, 2), (piece_k1, 3)],
                    9: [(piece_k2, 3), (piece_v1, 3)],
                    10: [(piece_v2, 3), (piece_tr, 3)],
                    11: [(piece_k1, 4), (piece_k2, 4)],
                    12: [(piece_v1, 4), (piece_v2, 4)],
                    13: [(piece_tr, 4), (piece_k1, 5)],
                    14: [(piece_k2, 5), (piece_v1, 5)],
                    15: [(piece_v2, 5), (piece_tr, 5)],
                    16: [(piece_k1, 6), (piece_k2, 6)],
                    17: [(piece_v1, 6), (piece_v2, 6)],
                    18: [(piece_tr, 6), (piece_k1, 7)],
                    19: [(piece_k2, 7), (piece_v1, 7)],
                    20: [(piece_v2, 7), (piece_tr, 7)],
                    21: [(piece_q1, 6), (piece_q2, 6)],
                    22: [(piece_q1, 7), (piece_q2, 7)],
                    23: [(piece_q1, 4), (piece_q2, 4)],
                    24: [(piece_q1, 5), (piece_q2, 5)],
                    25: [(piece_k2dma, 0), (piece_k2dma, 1)],
                },
                {
                    13: [(piece_q1, 2), (piece_q2, 2)],
                    15: [(piece_q1, 3), (piece_q2, 3)],
                },
                {},
            ]

            def _sph3_sched(tbl, chunks, jb0):
                """Schedule bcast + 6 outproj steps per chunk, 2 ops/jb."""
                jb = jb0
                for t in chunks:
                    tbl.setdefault(jb, []).append((sph3_bcast, t))
                    jb += 1
                    for fb0 in (0, 2, 4):
                        ops = tbl.setdefault(jb, [])
                        ops.append((lambda c, t=t, fb=fb0: sph3_step(t, fb), 0))
                        ops.append((lambda c, t=t, fb=fb0 + 1: sph3_step(t, fb), 0))
                        jb += 1

            _sph3_sched(inj[1], [0, 1], 5)
            _sph3_sched(inj[2], [4, 5], 5)
            _sph3_sched(inj[2], [6, 7], 20)

            def emit_av(pi, prev, c1, c2, ov):
                jb, etiles = prev
                for h, e in enumerate(etiles):
                    for ci, (_, ic, _, _, vt, m) in enumerate((c1, c2)):
                        nc.tensor.matmul(
                            ov[0:m, ci * W + h * 512:ci * W + (h + 1) * 512],
                            vt[:, jb, 0:m], e[:, ci * 512:(ci + 1) * 512],
                            start=(jb == 0), stop=(jb == NJB - 1))

            def drain(pi, c1, c2, ov):
                (n1, ic1, _, _, _, _), (n2, ic2, _, _, _, _) = c1, c2
                p10, p20 = ic1 * W, ic2 * W
                ovA, ovB = ov[:, 0:W], ov[:, W:2 * W]
                if n2 == "B":
                    ovb_sb = pe.tile([64, W], f32, tag="ovb_sb", name=f"ovb_sb{pi}", bufs=2)
                    nc.vector.tensor_copy(ovb_sb[:], ovB[0:64, :])
                    nc.gpsimd.dma_start(out=attn_out[64:128, p20:p20 + W], in_=ovb_sb[:])
                    nc.vector.tensor_copy(den[64:66, p20:p20 + W], ovB[64:66, :])
                    nc.vector.tensor_copy(den[64:65, p10:p10 + W], ovA[64:65, :])
                    if pi == len(pairs) - 1:
                        # last pair: ScalarE is idle after the final exp —
                        # give it the attn copy so den feeds the tail sooner
                        nc.scalar.copy(attn_out[0:64, p10:p10 + W], ovA[0:64, :])
                    else:
                        nc.vector.tensor_copy(attn_out[0:64, p10:p10 + W], ovA[0:64, :])
                else:
                    nc.vector.tensor_copy(den[64:65, p10:p10 + W], ovA[64:65, :])
                    nc.vector.tensor_copy(den[64:65, p20:p20 + W], ovB[64:65, :])
                    nc.vector.tensor_copy(attn_out[0:64, p10:p10 + W], ovA[0:64, :])
                    nc.vector.tensor_copy(attn_out[0:64, p20:p20 + W], ovB[0:64, :])

            for pi, (c1, c2) in enumerate(pairs):
                ov = ps_ov.tile([128, 2 * W], f32, tag="ov", name=f"ov_{pi}")
                prev_av = None
                for jb in range(NJB):
                    # scores + exp first: the exp stream must never wait on
                    # attn@V sitting ahead of it in the TensorE FIFO
                    etiles = []
                    for h in range(NH):
                        e = pre_e.pop((pi, jb, h), None)
                        if e is None:
                            e = _score_e(pi, jb, h, c1, c2)
                        etiles.append(e)
                    for fn, c in inj[pi].get(jb, ()):
                        fn(c)
                    if AV_LAG:
                        if prev_av is not None:
                            emit_av(pi, prev_av, c1, c2, ov)
                        prev_av = (jb, etiles)
                    else:
                        emit_av(pi, (jb, etiles), c1, c2, ov)
                        prev_av = None

                # ---- pair boundary: keep the exp stream running through the
                # drain by pre-emitting 3 blocks of next-pair scores/exp ----
                nxt = pairs[pi + 1] if pi + 1 < len(pairs) else None
                if nxt is not None:
                    for h in range(NH):
                        pre_e[(pi + 1, 0, h)] = _score_e(pi + 1, 0, h, *nxt)
                if prev_av is not None:
                    emit_av(pi, prev_av, c1, c2, ov)

                # drain accumulators to SBUF
                drain(pi, c1, c2, ov)

                if nxt is not None:
                    for h in range(NH):
                        pre_e[(pi + 1, 1, h)] = _score_e(pi + 1, 1, h, *nxt)
                    for h in range(NH):
                        pre_e[(pi + 1, 2, h)] = _score_e(pi + 1, 2, h, *nxt)

            # ---- tail: chunks 2,3 (out cols 1024:2048), 1024-wide ops ----
            nm23 = p3.tile([128, 1024], f16, tag="nm23", bufs=1)
            for idx, t in enumerate((2, 3)):
                t0 = t * 512
                bcg = ps_s.tile([128, 512], f32, tag="s", name=f"tbc{t}")
                nc.tensor.matmul(bcg[:], sel_sb[64:66, :], den[64:66, t0:t0 + 512],
                                 start=True, stop=True)
                rc = p3.tile([128, 512], f32, tag="rc", name=f"trc{t}", bufs=3)
                nc.vector.reciprocal_approx_fast(out=rc[:], in_=bcg[:])
                nc.vector.tensor_tensor(out=nm23[:, idx * 512:(idx + 1) * 512],
                                        in0=attn_out[:, t0:t0 + 512],
                                        in1=rc[:], op=MULT)
            ovt = ps_ov.tile([128, 2 * W], f32, tag="ov", name="tailov")
            for fb in range(6):
                if fb % 2 == 0:
                    po = ps_s.tile([128, 1024], f32, tag="s", name=f"tpo{fb}")
                else:
                    half = (fb // 2) % 2
                    po = ovt[:, half * 1024:(half + 1) * 1024]
                nc.tensor.matmul(po[:, 0:512], wo_sb[:, fb * 128:(fb + 1) * 128],
                                 nm23[:, 0:512], start=True, stop=True)
                nc.tensor.matmul(po[:, 512:1024], wo_sb[:, fb * 128:(fb + 1) * 128],
                                 nm23[:, 512:1024], start=True, stop=True)
                ob = p3.tile([128, 1024], f16, tag="ob2", name=f"tob{fb}", bufs=4)
                if fb % 2 == 1:
                    nc.scalar.copy(ob[:], po[:, 0:1024])
                else:
                    nc.vector.tensor_copy(ob[:], po[:, 0:1024])
                eng = nc.sync if fb % 2 == 0 else nc.gpsimd
                eng.dma_start(out=out_d[fb * 128:(fb + 1) * 128, 1024:2048],
                              in_=ob[:])

    nc.compile()
    return nc


def _get_program():
    global _PROGRAM
    if _PROGRAM is None:
        _PROGRAM = _build_program()
    return _PROGRAM


def kernel(x: np.ndarray, w_qkv: np.ndarray, w_out: np.ndarray) -> np.ndarray:
    global LAST_RESULT
    import os
    try:
        import antenv.axon_hooks  # noqa: F401
    except ImportError:
        # without the NTFF hook, a leaked BASS_TRACE=1 would crash the
        # axon trace path inside run_bass_kernel_spmd
        os.environ["BASS_NEVER_TRACE"] = "1"
    from concourse.bass_utils import run_bass_kernel_spmd

    nc = _get_program()
    x2 = np.ascontiguousarray(x[0], dtype=np.float32)          # [SEQ, F]
    w_qkv = np.asarray(w_qkv, dtype=np.float32)                # [2304, F]
    w_out = np.asarray(w_out, dtype=np.float32)                # [F, 768]

    # per-head slices of w_qkv rows: o = h*192 + d*3 + {0:q, 1:k, 2:v}
    def wslice(h, which):
        return w_qkv[h * 192 + which:(h + 1) * 192:3, :]       # [64, F]

    ident = np.eye(128, dtype=np.float16)
    sel = np.zeros((2, 128), dtype=np.float32)
    sel[0, 0:64] = 1.0
    sel[1, 64:128] = 1.0

    xt_plain = np.ascontiguousarray(x2.T.astype(np.float16))   # [F, SEQ]
    xt_rolled = np.ascontiguousarray(np.roll(x2, -SEQ // 2, axis=0).T.astype(np.float16))

    in_maps = []
    rolls = []
    for c in range(N_CORES):
        hA = c
        hB = 8 + c // 2
        roll = (SEQ // 2) if (c % 2) else 0
        rolls.append(roll)
        wqk = np.ascontiguousarray(np.concatenate(
            [wslice(hA, 0), wslice(hB, 0), wslice(hA, 1), wslice(hB, 1)],
            axis=0).T.astype(np.float16))
        wv = np.ascontiguousarray(np.concatenate(
            [wslice(hA, 2), wslice(hB, 2)], axis=0).T.astype(np.float16))
        cols = list(range(hA * 64, hA * 64 + 64)) + list(range(hB * 64, hB * 64 + 64))
        wo = np.ascontiguousarray(w_out[:, cols].T.astype(np.float16))  # [128, F]
        in_maps.append({
            "xt": xt_rolled if roll else xt_plain,
            "wqk": wqk, "wv": wv, "wo": wo, "sel": sel, "ident": ident,
        })

    res = run_bass_kernel_spmd(nc, in_maps, list(range(N_CORES)))
    LAST_RESULT = res

    acc = np.zeros((SEQ, N_FEATS), dtype=np.float64)
    for c in range(N_CORES):
        part = res.results[c]["out"]                           # [F, SEQ] fp16
        if rolls[c]:
            part = np.roll(part, rolls[c], axis=1)
        acc += part.T.astype(np.float64)
    return acc.astype(np.float32)[None]
